# revision 3
# baseline (speedup 1.0000x reference)
"""GCN layer (copy_src/sum message passing + Linear + ReLU) on 8 TRN2 cores.

    h[v] = sum_{(u,v) in E} feature[u];  out = relu(h @ W.T + b)

Strategy (1D dst partition, feature replicated):
- nodes sharded by dst across 8 cores (12500 rows each); each core owns the
  edges whose dst falls in its shard and produces its 12500x128 output slice.
- feature is replicated to every core as a bf16 table; per-edge rows are
  fetched with SWDGE dma_gather (int16 indices -> table split in 4 quarters
  of 25000 rows).
- scatter-add is a one-hot matmul: for each 128-node dst tile, chunks of 128
  edges are multiplied as X[e,f].T @ O[e,d] accumulating h^T[f,d] in PSUM
  (O one-hot built on DVE via iota==dstv compare, exact in bf16).
- per tile: h^T -> SBUF f32, f32 matmul with W^T, +bias, ReLU, DMA out.

Host prep chooses a schedule (run lengths per (tile, quarter)) shared by all
cores: L[t,q] = ceil128(max over cores of bucket size). Pad slots gather row 0
with a dead one-hot (dstv=200) so they contribute nothing.
"""

import numpy as np
import ml_dtypes

import concourse.bacc as bacc
import concourse.mybir as mybir
import concourse.tile as tile
from concourse.bass_utils import run_bass_kernel_spmd
from concourse.library_config import mlp

N_NODES = 100000
D = 128
N_CORES = 8
NC = N_NODES // N_CORES      # 12500 local nodes per core
NQ = 4                       # feature table quarters (int16 index range)
QROWS = N_NODES // NQ        # 25000
P = 128
T = (NC + P - 1) // P        # 98 dst tiles per core
PAD_DSTV = 200.0
GROUP_TILES = 3              # dst tiles per pipeline group
MAX_GATHER_IDXS = 1024       # per-instruction SWDGE gather cap (HW: idx
                             # pattern-read shape limit -> 64 cols of 16)

_f32 = mybir.dt.float32
_bf16 = mybir.dt.bfloat16
_i16 = mybir.dt.int16


def _ceil128(x):
    return max(128, -(-int(x) // 128) * 128)


def _wrap16(a):
    """[n] int -> [128, n/16] int16: idx i at [i%16, i//16], replicated x8."""
    w = a.reshape(-1, 16).T.astype(np.int16)
    return np.tile(w, (8, 1))


def _prep(src, dst):
    """Schedule + per-core index/dstv arrays. Schedule identical across cores."""
    core = dst // NC
    dstl = dst - core * NC
    tile_ = dstl // P
    q = src // QROWS

    key = (core * T + tile_) * NQ + q
    counts = np.bincount(key, minlength=N_CORES * T * NQ).reshape(N_CORES, T, NQ)
    cmax = counts.max(axis=0)  # [T, NQ]
    L = np.zeros((T, NQ), dtype=np.int64)
    for t in range(T):
        for qq in range(NQ):
            L[t, qq] = _ceil128(cmax[t, qq]) if cmax[t, qq] > 0 else 0
        if L[t].sum() == 0:
            L[t, 0] = 128  # guarantee >=1 chunk so PSUM gets a start matmul

    groups = [list(range(i, min(i + GROUP_TILES, T)))
              for i in range(0, T, GROUP_TILES)]

    slot_of = np.zeros((T, NQ), dtype=np.int64)
    ofs = 0
    for g in groups:
        for qq in range(NQ):
            for t in g:
                slot_of[t, qq] = ofs
                ofs += L[t, qq]
    S = ofs

    per_core = []
    for c in range(N_CORES):
        sel = core == c
        s_c, t_c, q_c, dl_c = src[sel], tile_[sel], q[sel], dstl[sel]
        order = np.lexsort((q_c, t_c))
        s_c, t_c, q_c, dl_c = s_c[order], t_c[order], q_c[order], dl_c[order]
        idx_slots = np.zeros(S, dtype=np.int16)
        dstv_slots = np.full(S, PAD_DSTV, dtype=np.float32)
        kk = t_c * NQ + q_c
        bounds = np.flatnonzero(np.diff(kk)) + 1
        starts = np.concatenate(([0], bounds))
        ends = np.concatenate((bounds, [len(kk)]))
        for a, b in zip(starts, ends):
            t, qq = int(t_c[a]), int(q_c[a])
            o = slot_of[t, qq]
            idx_slots[o:o + (b - a)] = (s_c[a:b] - qq * QROWS).astype(np.int16)
            dstv_slots[o:o + (b - a)] = (dl_c[a:b] - t * P).astype(np.float32)
        per_core.append({
            "idxs": _wrap16(idx_slots),
            "dstv": np.ascontiguousarray(
                dstv_slots.reshape(-1, P).T.astype(np.float32)),
        })
    return L, slot_of, S, groups, per_core


def _build(L, slot_of, S, groups):
    nc = bacc.Bacc("TRN2", target_bir_lowering=False, debug=False,
                   num_devices=N_CORES)
    table = nc.dram_tensor("table", [N_NODES, D], _bf16, kind="ExternalInput").ap()
    idxs_d = nc.dram_tensor("idxs", [128, S // 16], _i16, kind="ExternalInput").ap()
    dstv_d = nc.dram_tensor("dstv", [128, S // 128], _f32, kind="ExternalInput").ap()
    iota_d = nc.dram_tensor("iota", [128, 128], _bf16, kind="ExternalInput").ap()
    wt_d = nc.dram_tensor("wt", [128, 128], _f32, kind="ExternalInput").ap()
    bias_d = nc.dram_tensor("bias", [128, 128], _f32, kind="ExternalInput").ap()
    out_d = nc.dram_tensor("out", [NC, D], _f32, kind="ExternalOutput").ap()

    eq = mybir.AluOpType.is_equal
    add = mybir.AluOpType.add

    with tile.TileContext(nc) as tc:
        nc.gpsimd.load_library(mlp)
        with (
            tc.tile_pool(name="const", bufs=1) as cp,
            tc.tile_pool(name="xp", bufs=2) as xp,
            tc.tile_pool(name="dvp", bufs=2) as dvp,
            tc.tile_pool(name="idxp", bufs=4) as idxp,
            tc.tile_pool(name="op", bufs=4) as op_,
            tc.tile_pool(name="htp", bufs=2) as htp,
            tc.tile_pool(name="obp", bufs=2) as obp,
            tc.tile_pool(name="hps", bufs=2, space="PSUM") as hp,
            tc.tile_pool(name="o3ps", bufs=2, space="PSUM") as o3p,
        ):
            iota_t = cp.tile([128, 128], _bf16, tag="iota")
            nc.sync.dma_start(iota_t[:], iota_d[:])
            wt_t = cp.tile([128, 128], _f32, tag="wt")
            nc.sync.dma_start(wt_t[:], wt_d[:])
            bias_t = cp.tile([128, 128], _f32, tag="bias")
            nc.sync.dma_start(bias_t[:], bias_d[:])

            for g in groups:
                nch_g = sum(int(L[t, qq]) for t in g for qq in range(NQ)) // 128
                chunk0 = slot_of[g[0], 0] // 128  # group slots are contiguous
                X = xp.tile([128, nch_g, 128], _bf16, tag="X")
                dv = dvp.tile([128, nch_g], _f32, tag="dv")
                nc.sync.dma_start(dv[:], dstv_d[:, chunk0:chunk0 + nch_g])
                for qq in range(NQ):
                    n_gq = sum(int(L[t, qq]) for t in g)
                    if n_gq == 0:
                        continue
                    so = slot_of[g[0], qq]
                    for p0 in range(0, n_gq, MAX_GATHER_IDXS):
                        n_p = min(MAX_GATHER_IDXS, n_gq - p0)
                        sp = so + p0
                        it = idxp.tile([128, n_p // 16], _i16, tag="idx")
                        nc.sync.dma_start(
                            it[:], idxs_d[:, sp // 16: sp // 16 + n_p // 16])
                        cb = (sp // 128) - chunk0
                        nc.gpsimd.dma_gather(
                            X[:, cb:cb + n_p // 128, :],
                            table[qq * QROWS:(qq + 1) * QROWS, :],
                            it[:], n_p, n_p, D)

                hpt = hp.tile([128, len(g) * 128], _f32, tag="h")
                for tl, t in enumerate(g):
                    n_t = sum(int(L[t, qq]) for qq in range(NQ)) // 128
                    ci = 0
                    for qq in range(NQ):
                        if L[t, qq] == 0:
                            continue
                        cb = (slot_of[t, qq] // 128) - chunk0
                        for ch in range(int(L[t, qq]) // 128):
                            gc = cb + ch
                            O = op_.tile([128, 128], _bf16, tag="O")
                            nc.vector.tensor_scalar(
                                O[:], iota_t[:], dv[:, gc:gc + 1], None, eq)
                            nc.tensor.matmul(
                                hpt[:, tl * 128:(tl + 1) * 128],
                                lhsT=X[:, gc, :], rhs=O[:],
                                start=(ci == 0), stop=(ci == n_t - 1))
                            ci += 1
                    ht = htp.tile([128, 128], _f32, tag="ht")
                    nc.scalar.copy(ht[:], hpt[:, tl * 128:(tl + 1) * 128])
                    o3 = o3p.tile([128, 128], _f32, tag="o3")
                    nc.tensor.matmul(o3[:], lhsT=ht[:], rhs=wt_t[:],
                                     start=True, stop=True)
                    ob = obp.tile([128, 128], _f32, tag="ob")
                    nc.vector.tensor_tensor(ob[:], o3[:], bias_t[:], op=add)
                    nc.vector.tensor_scalar_max(ob[:], ob[:], 0.0)
                    r0 = t * P
                    nrows = min(P, NC - r0)
                    nc.sync.dma_start(out_d[r0:r0 + nrows, :], ob[:nrows, :])
    nc.compile()
    return nc


_CACHE = {}
LAST_RES = None  # full BassKernelResults of the most recent run (for test.py)


def _get_compiled(src, dst):
    key = (hash(src.tobytes()), hash(dst.tobytes()))
    if key not in _CACHE:
        L, slot_of, S, groups, per_core = _prep(src, dst)
        nc = _build(L, slot_of, S, groups)
        _CACHE.clear()
        _CACHE[key] = (nc, per_core)
    return _CACHE[key]


def _run(feature, src, dst, W, b, trace=False):
    feature = np.asarray(feature, dtype=np.float32)
    src = np.asarray(src).astype(np.int64)
    dst = np.asarray(dst).astype(np.int64)
    W = np.asarray(W, dtype=np.float32)
    b = np.asarray(b, dtype=np.float32)

    nc, per_core = _get_compiled(src, dst)

    table = feature.astype(ml_dtypes.bfloat16)
    iota = np.tile(np.arange(128, dtype=np.float32), (128, 1)).astype(
        ml_dtypes.bfloat16)
    wt = np.ascontiguousarray(W.T)           # [in, out]
    bias = np.tile(b, (128, 1)).astype(np.float32)

    in_maps = []
    for c in range(N_CORES):
        in_maps.append({
            "table": table,
            "idxs": per_core[c]["idxs"],
            "dstv": per_core[c]["dstv"],
            "iota": iota,
            "wt": wt,
            "bias": bias,
        })
    res = run_bass_kernel_spmd(nc, in_maps, core_ids=list(range(N_CORES)),
                               trace=trace)
    global LAST_RES
    LAST_RES = res
    out = np.concatenate([res.results[c]["out"] for c in range(N_CORES)], axis=0)
    return out.astype(np.float32), res.exec_time_ns


def kernel(feature, src, dst, W, b):
    return _run(feature, src, dst, W, b)[0]


def timed_run(inputs):
    return _run(**inputs, trace=True)[1]



# revision 4
# speedup vs baseline: 1.1995x; 1.1995x over previous
"""GCN layer (copy_src/sum message passing + Linear + ReLU) on 8 TRN2 cores.

    h[v] = sum_{(u,v) in E} feature[u];  out = relu(h @ W.T + b)

Strategy (1D dst partition, feature replicated):
- nodes sharded by dst across 8 cores (12500 rows each); each core owns the
  edges whose dst falls in its shard and produces its 12500x128 output slice.
- feature is replicated to every core as a bf16 table; per-edge rows are
  fetched with SWDGE dma_gather (int16 indices -> table split in 4 quarters
  of 25000 rows).
- scatter-add is a one-hot matmul: for each 128-node dst tile, chunks of 128
  edges are multiplied as X[e,f].T @ O[e,d] accumulating h^T[f,d] in PSUM
  (O one-hot built on DVE via iota==dstv compare, exact in bf16).
- per tile: h^T -> SBUF f32, f32 matmul with W^T, +bias, ReLU, DMA out.

Host prep chooses a schedule (run lengths per (tile, quarter)) shared by all
cores: L[t,q] = ceil128(max over cores of bucket size). Pad slots gather row 0
with a dead one-hot (dstv=200) so they contribute nothing.

Perf notes (HW-measured 2026-08-09, see /tmp/last_trace + bench_gather.py):
- The kernel is bound by SWDGE dma_gather Q7 descriptor GENERATION on the
  POOL engine: ~8.6-11 ns per gathered row, ~10.3us per 1024-idx gather
  instruction; POOL is ~100% busy for the whole span. SDMA drain is only
  ~1.9us per gather (22% active); HBM bandwidth is NOT the bottleneck.
- num_idxs > 1024 (e.g. 2048) CRASHES the device (idx pattern-read limit of
  64 cols x 16 is a hard HW/ucode limit).
- ap_gather (lib 6) silently no-ops on this deployment (wrong results,
  ~510ns flat); indirect_copy hangs the device. No Q7 SBUF-gather available.
- Interleaved/trailing negative-idx skip and sbuf-source/transpose gather
  variants crashed the device in bench runs; not usable here.
- Floor for this architecture: slots (~451k/core) x ~10ns ~= 4.4ms. Only
  ~5-10% is recoverable (ceil128 padding trim, window tail merges).
"""

import numpy as np
import ml_dtypes

import concourse.bacc as bacc
import concourse.mybir as mybir
import concourse.tile as tile
from concourse.bass_utils import run_bass_kernel_spmd
from concourse.library_config import mlp

N_NODES = 100000
D = 128
N_CORES = 8
NC = N_NODES // N_CORES      # 12500 local nodes per core
NQ = 4                       # feature table quarters (int16 index range)
QROWS = N_NODES // NQ        # 25000
P = 128
T = (NC + P - 1) // P        # 98 dst tiles per core
PAD_DSTV = 200.0
GROUP_TILES = 3              # dst tiles per pipeline group
MAX_GATHER_IDXS = 1024       # per-instruction SWDGE gather cap (HW: idx
                             # pattern-read shape limit -> 64 cols of 16)

_f32 = mybir.dt.float32
_bf16 = mybir.dt.bfloat16
_i16 = mybir.dt.int16


def _ceil128(x):
    return max(128, -(-int(x) // 128) * 128)


def _wrap16(a):
    """[n] int -> [128, n/16] int16: idx i at [i%16, i//16], replicated x8."""
    w = a.reshape(-1, 16).T.astype(np.int16)
    return np.tile(w, (8, 1))


def _prep(src, dst):
    """Schedule + per-core index/dstv arrays. Schedule identical across cores."""
    core = dst // NC
    dstl = dst - core * NC
    tile_ = dstl // P
    q = src // QROWS

    key = (core * T + tile_) * NQ + q
    counts = np.bincount(key, minlength=N_CORES * T * NQ).reshape(N_CORES, T, NQ)
    cmax = counts.max(axis=0)  # [T, NQ]
    L = np.zeros((T, NQ), dtype=np.int64)
    for t in range(T):
        for qq in range(NQ):
            L[t, qq] = _ceil128(cmax[t, qq]) if cmax[t, qq] > 0 else 0
        if L[t].sum() == 0:
            L[t, 0] = 128  # guarantee >=1 chunk so PSUM gets a start matmul

    groups = [list(range(i, min(i + GROUP_TILES, T)))
              for i in range(0, T, GROUP_TILES)]

    slot_of = np.zeros((T, NQ), dtype=np.int64)
    ofs = 0
    for g in groups:
        for qq in range(NQ):
            for t in g:
                slot_of[t, qq] = ofs
                ofs += L[t, qq]
    S = ofs

    per_core = []
    for c in range(N_CORES):
        sel = core == c
        s_c, t_c, q_c, dl_c = src[sel], tile_[sel], q[sel], dstl[sel]
        order = np.lexsort((q_c, t_c))
        s_c, t_c, q_c, dl_c = s_c[order], t_c[order], q_c[order], dl_c[order]
        idx_slots = np.zeros(S, dtype=np.int16)
        dstv_slots = np.full(S, PAD_DSTV, dtype=np.float32)
        kk = t_c * NQ + q_c
        bounds = np.flatnonzero(np.diff(kk)) + 1
        starts = np.concatenate(([0], bounds))
        ends = np.concatenate((bounds, [len(kk)]))
        for a, b in zip(starts, ends):
            t, qq = int(t_c[a]), int(q_c[a])
            o = slot_of[t, qq]
            idx_slots[o:o + (b - a)] = (s_c[a:b] - qq * QROWS).astype(np.int16)
            dstv_slots[o:o + (b - a)] = (dl_c[a:b] - t * P).astype(np.float32)
        per_core.append({
            "idxs": _wrap16(idx_slots),
            "dstv": np.ascontiguousarray(
                dstv_slots.reshape(-1, P).T.astype(np.float32)),
        })
    return L, slot_of, S, groups, per_core


def _build(L, slot_of, S, groups):
    nc = bacc.Bacc("TRN2", target_bir_lowering=False, debug=False,
                   num_devices=N_CORES)
    table = nc.dram_tensor("table", [N_NODES, D], _bf16, kind="ExternalInput").ap()
    idxs_d = nc.dram_tensor("idxs", [128, S // 16], _i16, kind="ExternalInput").ap()
    dstv_d = nc.dram_tensor("dstv", [128, S // 128], _f32, kind="ExternalInput").ap()
    iota_d = nc.dram_tensor("iota", [128, 128], _bf16, kind="ExternalInput").ap()
    wt_d = nc.dram_tensor("wt", [128, 128], _f32, kind="ExternalInput").ap()
    bias_d = nc.dram_tensor("bias", [128, 128], _f32, kind="ExternalInput").ap()
    out_d = nc.dram_tensor("out", [NC, D], _f32, kind="ExternalOutput").ap()

    eq = mybir.AluOpType.is_equal
    add = mybir.AluOpType.add

    with tile.TileContext(nc) as tc:
        nc.gpsimd.load_library(mlp)
        with (
            tc.tile_pool(name="const", bufs=1) as cp,
            tc.tile_pool(name="xp", bufs=2) as xp,
            tc.tile_pool(name="dvp", bufs=2) as dvp,
            tc.tile_pool(name="idxp", bufs=4) as idxp,
            tc.tile_pool(name="op", bufs=4) as op_,
            tc.tile_pool(name="htp", bufs=2) as htp,
            tc.tile_pool(name="obp", bufs=2) as obp,
            tc.tile_pool(name="hps", bufs=2, space="PSUM") as hp,
            tc.tile_pool(name="o3ps", bufs=2, space="PSUM") as o3p,
        ):
            iota_t = cp.tile([128, 128], _bf16, tag="iota")
            nc.sync.dma_start(iota_t[:], iota_d[:])
            wt_t = cp.tile([128, 128], _f32, tag="wt")
            nc.sync.dma_start(wt_t[:], wt_d[:])
            bias_t = cp.tile([128, 128], _f32, tag="bias")
            nc.sync.dma_start(bias_t[:], bias_d[:])

            for g in groups:
                nch_g = sum(int(L[t, qq]) for t in g for qq in range(NQ)) // 128
                chunk0 = slot_of[g[0], 0] // 128  # group slots are contiguous
                X = xp.tile([128, nch_g, 128], _bf16, tag="X")
                dv = dvp.tile([128, nch_g], _f32, tag="dv")
                nc.sync.dma_start(dv[:], dstv_d[:, chunk0:chunk0 + nch_g])
                for qq in range(NQ):
                    n_gq = sum(int(L[t, qq]) for t in g)
                    if n_gq == 0:
                        continue
                    so = slot_of[g[0], qq]
                    for p0 in range(0, n_gq, MAX_GATHER_IDXS):
                        n_p = min(MAX_GATHER_IDXS, n_gq - p0)
                        sp = so + p0
                        it = idxp.tile([128, n_p // 16], _i16, tag="idx")
                        nc.sync.dma_start(
                            it[:], idxs_d[:, sp // 16: sp // 16 + n_p // 16])
                        cb = (sp // 128) - chunk0
                        nc.gpsimd.dma_gather(
                            X[:, cb:cb + n_p // 128, :],
                            table[qq * QROWS:(qq + 1) * QROWS, :],
                            it[:], n_p, n_p, D)

                hpt = hp.tile([128, len(g) * 128], _f32, tag="h")
                for tl, t in enumerate(g):
                    n_t = sum(int(L[t, qq]) for qq in range(NQ)) // 128
                    ci = 0
                    for qq in range(NQ):
                        if L[t, qq] == 0:
                            continue
                        cb = (slot_of[t, qq] // 128) - chunk0
                        for ch in range(int(L[t, qq]) // 128):
                            gc = cb + ch
                            O = op_.tile([128, 128], _bf16, tag="O")
                            nc.vector.tensor_scalar(
                                O[:], iota_t[:], dv[:, gc:gc + 1], None, eq)
                            nc.tensor.matmul(
                                hpt[:, tl * 128:(tl + 1) * 128],
                                lhsT=X[:, gc, :], rhs=O[:],
                                start=(ci == 0), stop=(ci == n_t - 1))
                            ci += 1
                    ht = htp.tile([128, 128], _f32, tag="ht")
                    nc.scalar.copy(ht[:], hpt[:, tl * 128:(tl + 1) * 128])
                    o3 = o3p.tile([128, 128], _f32, tag="o3")
                    nc.tensor.matmul(o3[:], lhsT=ht[:], rhs=wt_t[:],
                                     start=True, stop=True)
                    ob = obp.tile([128, 128], _f32, tag="ob")
                    nc.vector.tensor_tensor(ob[:], o3[:], bias_t[:], op=add)
                    nc.vector.tensor_scalar_max(ob[:], ob[:], 0.0)
                    r0 = t * P
                    nrows = min(P, NC - r0)
                    nc.sync.dma_start(out_d[r0:r0 + nrows, :], ob[:nrows, :])
    nc.compile()
    return nc


_CACHE = {}
LAST_RES = None  # full BassKernelResults of the most recent run (for test.py)


def _get_compiled(src, dst):
    key = (hash(src.tobytes()), hash(dst.tobytes()))
    if key not in _CACHE:
        L, slot_of, S, groups, per_core = _prep(src, dst)
        nc = _build(L, slot_of, S, groups)
        _CACHE.clear()
        _CACHE[key] = (nc, per_core)
    return _CACHE[key]


def _run(feature, src, dst, W, b, trace=False):
    feature = np.asarray(feature, dtype=np.float32)
    src = np.asarray(src).astype(np.int64)
    dst = np.asarray(dst).astype(np.int64)
    W = np.asarray(W, dtype=np.float32)
    b = np.asarray(b, dtype=np.float32)

    nc, per_core = _get_compiled(src, dst)

    table = feature.astype(ml_dtypes.bfloat16)
    iota = np.tile(np.arange(128, dtype=np.float32), (128, 1)).astype(
        ml_dtypes.bfloat16)
    wt = np.ascontiguousarray(W.T)           # [in, out]
    bias = np.tile(b, (128, 1)).astype(np.float32)

    in_maps = []
    for c in range(N_CORES):
        in_maps.append({
            "table": table,
            "idxs": per_core[c]["idxs"],
            "dstv": per_core[c]["dstv"],
            "iota": iota,
            "wt": wt,
            "bias": bias,
        })
    res = run_bass_kernel_spmd(nc, in_maps, core_ids=list(range(N_CORES)),
                               trace=trace)
    global LAST_RES
    LAST_RES = res
    out = np.concatenate([res.results[c]["out"] for c in range(N_CORES)], axis=0)
    return out.astype(np.float32), res.exec_time_ns


def kernel(feature, src, dst, W, b):
    return _run(feature, src, dst, W, b)[0]


def timed_run(inputs):
    return _run(**inputs, trace=True)[1]



# revision 6
# speedup vs baseline: 1.2735x; 1.0616x over previous
"""GCN layer on 8 TRN2 cores — v2: concat-bucket schedule (no per-bucket
ceil128 padding) + 6-tile groups (fewer gather-window tails).

Same architecture as v1 (dst-sharded, replicated bf16 table, SWDGE
dma_gather per (group, quarter), one-hot scatter matmuls), but the slot
grid packs each (group, quarter) run as the concatenation of raw
max-over-cores bucket lengths (128-clamped), padded to 128 only per run.
Chunks that straddle a bucket boundary are processed twice with two
one-hot "role" columns (dv2[:, 2c] / dv2[:, 2c+1]); dead rows carry
dstv=200. Gathers fetch only ceil16(valid) slots per run; the stale tail
of the last chunk is zeroed by the one-hot.
"""

import numpy as np
import ml_dtypes

import concourse.bacc as bacc
import concourse.mybir as mybir
import concourse.tile as tile
from concourse.bass_utils import run_bass_kernel_spmd
from concourse.library_config import mlp

N_NODES = 100000
D = 128
N_CORES = 8
NC = N_NODES // N_CORES      # 12500 local nodes per core
NQ = 4                       # feature table quarters (int16 index range)
QROWS = N_NODES // NQ        # 25000
P = 128
T = (NC + P - 1) // P        # 98 dst tiles per core
PAD_DSTV = 200.0
GROUP_TILES = 6
MAX_GATHER_IDXS = 1024       # hard HW cap (64 idx cols x 16)

_f32 = mybir.dt.float32
_bf16 = mybir.dt.bfloat16
_i16 = mybir.dt.int16


def _ceil(x, m):
    return -(-int(x) // m) * m


def _wrap16(a):
    """[n] int -> [128, n/16] int16: idx i at [i%16, i//16], replicated x8."""
    w = a.reshape(-1, 16).T.astype(np.int16)
    return np.tile(w, (8, 1))


def _prep(src, dst):
    """Concat-bucket schedule shared by all cores + per-core idx/dv2 arrays.

    Returns meta dict with:
      groups: list of tile lists
      run_start[g][q]: global slot offset of (g,q) run (128-aligned)
      run_len[g][q]: run length in slots (128-mult)
      run_valid16[g][q]: gathered slot count (16-mult, <= run_len)
      tile_ops[t]: ordered list of (chunk_global, role) for tile t
      n_chunks: total chunks
      S: total slots (128-mult)
    and per-core dicts: idxs [128, S/16] int16, dv2 [128, 2*n_chunks] f32.
    """
    core = dst // NC
    dstl = dst - core * NC
    tile_ = dstl // P
    q = src // QROWS

    key = (core * T + tile_) * NQ + q
    counts = np.bincount(key, minlength=N_CORES * T * NQ).reshape(N_CORES, T, NQ)
    cmax = counts.max(axis=0)  # [T, NQ]

    Lraw = np.zeros((T, NQ), dtype=np.int64)
    for t in range(T):
        for qq in range(NQ):
            c = int(cmax[t, qq])
            Lraw[t, qq] = 0 if c == 0 else max(128, c)
        if Lraw[t].sum() == 0:
            Lraw[t, 0] = 128  # guarantee >=1 chunk so PSUM gets a start matmul

    groups = [list(range(i, min(i + GROUP_TILES, T)))
              for i in range(0, T, GROUP_TILES)]

    nG = len(groups)
    run_start = [[0] * NQ for _ in range(nG)]
    run_len = [[0] * NQ for _ in range(nG)]
    run_valid16 = [[0] * NQ for _ in range(nG)]
    bucket_ofs = np.zeros((T, NQ), dtype=np.int64)  # global slot offset
    ofs = 0
    for gi, g in enumerate(groups):
        for qq in range(NQ):
            run_start[gi][qq] = ofs
            cur = ofs
            for t in g:
                bucket_ofs[t, qq] = cur
                cur += Lraw[t, qq]
            valid = cur - ofs
            # gather windows must be multiples of 128 rows: a window with
            # <128 rows leaves SDMA engines descriptor-less -> sem hang
            run_len[gi][qq] = _ceil(valid, 128)
            run_valid16[gi][qq] = run_len[gi][qq]
            ofs += run_len[gi][qq]
    S = ofs
    n_chunks = S // 128

    # chunk -> list of (tile, role); tile -> list of (chunk, role)
    tile_ops = [[] for _ in range(T)]
    chunk_roles = [[] for _ in range(n_chunks)]
    for gi, g in enumerate(groups):
        for qq in range(NQ):
            for t in g:
                L = Lraw[t, qq]
                if L == 0:
                    continue
                o = bucket_ofs[t, qq]
                c0, c1 = o // 128, (o + L - 1) // 128
                for c in range(c0, c1 + 1):
                    r = len(chunk_roles[c])
                    assert r < 2, "bucket <128 slots broke 2-role invariant"
                    chunk_roles[c].append(t)
                    tile_ops[t].append((int(c), r))

    per_core = []
    for c in range(N_CORES):
        sel = core == c
        s_c, t_c, q_c, dl_c = src[sel], tile_[sel], q[sel], dstl[sel]
        order = np.lexsort((q_c, t_c))
        s_c, t_c, q_c, dl_c = s_c[order], t_c[order], q_c[order], dl_c[order]
        idx_slots = np.zeros(S, dtype=np.int16)
        dstv_slots = np.full(S, PAD_DSTV, dtype=np.float32)
        owner = np.full(S, -1, dtype=np.int64)  # which tile owns slot (pads too)
        for t in range(T):
            for qq in range(NQ):
                if Lraw[t, qq]:
                    o = bucket_ofs[t, qq]
                    owner[o:o + Lraw[t, qq]] = t
        kk = t_c * NQ + q_c
        bounds = np.flatnonzero(np.diff(kk)) + 1
        starts = np.concatenate(([0], bounds))
        ends = np.concatenate((bounds, [len(kk)]))
        for a, b in zip(starts, ends):
            t, qq = int(t_c[a]), int(q_c[a])
            o = bucket_ofs[t, qq]
            n = b - a
            assert n <= Lraw[t, qq]
            idx_slots[o:o + n] = (s_c[a:b] - qq * QROWS).astype(np.int16)
            dstv_slots[o:o + n] = (dl_c[a:b] - t * P).astype(np.float32)
        # dv2: [128, 2*n_chunks]: col 2c+r = dstv where slot owner == role-r
        # tile of chunk c, else 200
        dv2 = np.full((128, 2 * n_chunks), PAD_DSTV, dtype=np.float32)
        sv = dstv_slots.reshape(n_chunks, 128)
        ow = owner.reshape(n_chunks, 128)
        for ch in range(n_chunks):
            for r, t in enumerate(chunk_roles[ch]):
                m = ow[ch] == t
                dv2[m, 2 * ch + r] = sv[ch][m]
        per_core.append({"idxs": _wrap16(idx_slots), "dv2": dv2})

    meta = {
        "groups": groups, "run_start": run_start, "run_len": run_len,
        "run_valid16": run_valid16, "tile_ops": tile_ops,
        "n_chunks": n_chunks, "S": S, "Lraw": Lraw,
    }
    return meta, per_core


def _host_check(meta, per_core, feature, src, dst, W, b):
    """Numpy simulation of the device pipeline; returns rel err vs reference."""
    table = feature.astype(ml_dtypes.bfloat16).astype(np.float32)
    outs = []
    for c in range(N_CORES):
        idxs = per_core[c]["idxs"]
        dv2 = per_core[c]["dv2"]
        # unwrap idx slots
        S = meta["S"]
        unw = idxs[:16, :].T.reshape(-1)[:S].astype(np.int64)
        X = np.zeros((S, D), dtype=np.float32)
        for gi, g in enumerate(meta["groups"]):
            for qq in range(NQ):
                st = meta["run_start"][gi][qq]
                v16 = meta["run_valid16"][gi][qq]
                rows = unw[st:st + v16] + qq * QROWS
                X[st:st + v16] = table[rows]
        h = np.zeros((T * 128, D), dtype=np.float32)
        for t in range(T):
            acc = np.zeros((128, D), dtype=np.float32)
            for (ch, r) in meta["tile_ops"][t]:
                o = dv2[:, 2 * ch + r]  # [128] dst-within-tile or 200
                Ochunk = (o[:, None] == np.arange(128)[None, :])
                acc += (X[ch * 128:(ch + 1) * 128].T.astype(ml_dtypes.bfloat16)
                        .astype(np.float32) @ Ochunk).T
            h[t * 128:(t + 1) * 128] = acc
        o3 = h[:NC] @ W.T + b
        outs.append(np.maximum(o3, 0.0))
    return np.concatenate(outs, axis=0)


def _build(meta):
    nc = bacc.Bacc("TRN2", target_bir_lowering=False, debug=False,
                   num_devices=N_CORES)
    n_chunks = meta["n_chunks"]
    S = meta["S"]
    table = nc.dram_tensor("table", [N_NODES, D], _bf16, kind="ExternalInput").ap()
    idxs_d = nc.dram_tensor("idxs", [128, S // 16], _i16, kind="ExternalInput").ap()
    dv2_d = nc.dram_tensor("dv2", [128, 2 * n_chunks], _f32,
                           kind="ExternalInput").ap()
    iota_d = nc.dram_tensor("iota", [128, 128], _bf16, kind="ExternalInput").ap()
    wt_d = nc.dram_tensor("wt", [128, 128], _f32, kind="ExternalInput").ap()
    bias_d = nc.dram_tensor("bias", [128, 128], _f32, kind="ExternalInput").ap()
    out_d = nc.dram_tensor("out", [NC, D], _f32, kind="ExternalOutput").ap()

    eq = mybir.AluOpType.is_equal
    add = mybir.AluOpType.add

    groups = meta["groups"]
    run_start = meta["run_start"]
    run_len = meta["run_len"]
    run_valid16 = meta["run_valid16"]
    tile_ops = meta["tile_ops"]

    with tile.TileContext(nc) as tc:
        nc.gpsimd.load_library(mlp)
        with (
            tc.tile_pool(name="const", bufs=1) as cp,
            tc.tile_pool(name="xp", bufs=2) as xp,
            tc.tile_pool(name="dvp", bufs=2) as dvp,
            tc.tile_pool(name="idxp", bufs=4) as idxp,
            tc.tile_pool(name="op", bufs=4) as op_,
            tc.tile_pool(name="htp", bufs=2) as htp,
            tc.tile_pool(name="obp", bufs=2) as obp,
            tc.tile_pool(name="hps", bufs=2, space="PSUM") as hp,
            tc.tile_pool(name="o3ps", bufs=2, space="PSUM") as o3p,
        ):
            iota_t = cp.tile([128, 128], _bf16, tag="iota")
            nc.sync.dma_start(iota_t[:], iota_d[:])
            wt_t = cp.tile([128, 128], _f32, tag="wt")
            nc.sync.dma_start(wt_t[:], wt_d[:])
            bias_t = cp.tile([128, 128], _f32, tag="bias")
            nc.sync.dma_start(bias_t[:], bias_d[:])

            for gi, g in enumerate(groups):
                g0 = run_start[gi][0]               # group slot base
                nch_g = sum(run_len[gi][qq] for qq in range(NQ)) // 128
                chunk0 = g0 // 128
                X = xp.tile([128, nch_g, 128], _bf16, tag="X")
                dv = dvp.tile([128, 2 * nch_g], _f32, tag="dv")
                nc.sync.dma_start(
                    dv[:], dv2_d[:, 2 * chunk0: 2 * (chunk0 + nch_g)])
                for qq in range(NQ):
                    v16 = run_valid16[gi][qq]
                    if v16 == 0:
                        continue
                    st = run_start[gi][qq]
                    for p0 in range(0, v16, MAX_GATHER_IDXS):
                        n_p = min(MAX_GATHER_IDXS, v16 - p0)
                        sp = st + p0
                        it = idxp.tile([128, n_p // 16], _i16, tag="idx")
                        nc.sync.dma_start(
                            it[:], idxs_d[:, sp // 16: sp // 16 + n_p // 16])
                        cb = (sp - g0) // 128
                        ncw = _ceil(n_p, 128) // 128
                        nc.gpsimd.dma_gather(
                            X[:, cb:cb + ncw, :],
                            table[qq * QROWS:(qq + 1) * QROWS, :],
                            it[:], n_p, n_p, D)

                hpt = hp.tile([128, len(g) * 128], _f32, tag="h")
                for tl, t in enumerate(g):
                    ops = tile_ops[t]
                    for oi, (ch, r) in enumerate(ops):
                        gc = ch - chunk0
                        O = op_.tile([128, 128], _bf16, tag="O")
                        nc.vector.tensor_scalar(
                            O[:], iota_t[:], dv[:, 2 * gc + r: 2 * gc + r + 1],
                            None, eq)
                        nc.tensor.matmul(
                            hpt[:, tl * 128:(tl + 1) * 128],
                            lhsT=X[:, gc, :], rhs=O[:],
                            start=(oi == 0), stop=(oi == len(ops) - 1))
                    ht = htp.tile([128, 128], _f32, tag="ht")
                    nc.scalar.copy(ht[:], hpt[:, tl * 128:(tl + 1) * 128])
                    o3 = o3p.tile([128, 128], _f32, tag="o3")
                    nc.tensor.matmul(o3[:], lhsT=ht[:], rhs=wt_t[:],
                                     start=True, stop=True)
                    ob = obp.tile([128, 128], _f32, tag="ob")
                    nc.vector.tensor_tensor(ob[:], o3[:], bias_t[:], op=add)
                    nc.vector.tensor_scalar_max(ob[:], ob[:], 0.0)
                    r0 = t * P
                    nrows = min(P, NC - r0)
                    nc.sync.dma_start(out_d[r0:r0 + nrows, :], ob[:nrows, :])
    nc.compile()
    return nc


_CACHE = {}
LAST_RES = None


def _get_compiled(src, dst):
    key = (hash(src.tobytes()), hash(dst.tobytes()))
    if key not in _CACHE:
        meta, per_core = _prep(src, dst)
        nc = _build(meta)
        _CACHE.clear()
        _CACHE[key] = (nc, meta, per_core)
    return _CACHE[key]


def _run(feature, src, dst, W, b, trace=False):
    feature = np.asarray(feature, dtype=np.float32)
    src = np.asarray(src).astype(np.int64)
    dst = np.asarray(dst).astype(np.int64)
    W = np.asarray(W, dtype=np.float32)
    b = np.asarray(b, dtype=np.float32)

    nc, meta, per_core = _get_compiled(src, dst)

    table = feature.astype(ml_dtypes.bfloat16)
    iota = np.tile(np.arange(128, dtype=np.float32), (128, 1)).astype(
        ml_dtypes.bfloat16)
    wt = np.ascontiguousarray(W.T)
    bias = np.tile(b, (128, 1)).astype(np.float32)

    in_maps = []
    for c in range(N_CORES):
        in_maps.append({
            "table": table,
            "idxs": per_core[c]["idxs"],
            "dv2": per_core[c]["dv2"],
            "iota": iota,
            "wt": wt,
            "bias": bias,
        })
    res = run_bass_kernel_spmd(nc, in_maps, core_ids=list(range(N_CORES)),
                               trace=trace)
    global LAST_RES
    LAST_RES = res
    out = np.concatenate([res.results[c]["out"] for c in range(N_CORES)], axis=0)
    return out.astype(np.float32), res.exec_time_ns


def kernel(feature, src, dst, W, b):
    return _run(feature, src, dst, W, b)[0]


def timed_run(inputs):
    return _run(**inputs, trace=True)[1]


# revision 7
# speedup vs baseline: 1.2833x; 1.0077x over previous
"""GCN layer on 8 TRN2 cores — v2: concat-bucket schedule (no per-bucket
ceil128 padding) + 6-tile groups (fewer gather-window tails).

Same architecture as v1 (dst-sharded, replicated bf16 table, SWDGE
dma_gather per (group, quarter), one-hot scatter matmuls), but the slot
grid packs each (group, quarter) run as the concatenation of raw
max-over-cores bucket lengths (128-clamped), padded to 128 only per run.
Chunks that straddle a bucket boundary are processed twice with two
one-hot "role" columns (dv2[:, 2c] / dv2[:, 2c+1]); dead rows carry
dstv=200. Gathers fetch only ceil16(valid) slots per run; the stale tail
of the last chunk is zeroed by the one-hot.
"""

import numpy as np
import ml_dtypes

import concourse.bacc as bacc
import concourse.mybir as mybir
import concourse.tile as tile
from concourse.bass_utils import run_bass_kernel_spmd
from concourse.library_config import mlp

N_NODES = 100000
D = 128
N_CORES = 8
NC = N_NODES // N_CORES      # 12500 local nodes per core
NQ = 4                       # feature table quarters (int16 index range)
QROWS = N_NODES // NQ        # 25000
P = 128
T = (NC + P - 1) // P        # 98 dst tiles per core
PAD_DSTV = 200.0
GROUP_TILES = 7
MAX_GATHER_IDXS = 1024       # hard HW cap (64 idx cols x 16)

_f32 = mybir.dt.float32
_bf16 = mybir.dt.bfloat16
_i16 = mybir.dt.int16


def _ceil(x, m):
    return -(-int(x) // m) * m


def _wrap16(a):
    """[n] int -> [128, n/16] int16: idx i at [i%16, i//16], replicated x8."""
    w = a.reshape(-1, 16).T.astype(np.int16)
    return np.tile(w, (8, 1))


def _prep(src, dst):
    """Concat-bucket schedule shared by all cores + per-core idx/dv2 arrays.

    Returns meta dict with:
      groups: list of tile lists
      run_start[g][q]: global slot offset of (g,q) run (128-aligned)
      run_len[g][q]: run length in slots (128-mult)
      run_valid16[g][q]: gathered slot count (16-mult, <= run_len)
      tile_ops[t]: ordered list of (chunk_global, role) for tile t
      n_chunks: total chunks
      S: total slots (128-mult)
    and per-core dicts: idxs [128, S/16] int16, dv2 [128, 2*n_chunks] f32.
    """
    core = dst // NC
    dstl = dst - core * NC
    tile_ = dstl // P
    q = src // QROWS

    key = (core * T + tile_) * NQ + q
    counts = np.bincount(key, minlength=N_CORES * T * NQ).reshape(N_CORES, T, NQ)
    # per-core tile permutation: sort each core's tiles by total edge count so
    # slot k pairs each core's k-th busiest tile -> max-over-cores shrinks
    totals = counts.sum(axis=2)                      # [8, T]
    perm = np.argsort(-totals, axis=1, kind="stable")  # [8, T] slot -> tile
    inv_perm = np.argsort(perm, axis=1)              # [8, T] tile -> slot
    cnt2 = np.take_along_axis(counts, perm[:, :, None], axis=1)  # [8, slot, q]
    cmax = cnt2.max(axis=0)  # [slot, NQ]

    Lraw = np.zeros((T, NQ), dtype=np.int64)
    for t in range(T):
        for qq in range(NQ):
            c = int(cmax[t, qq])
            Lraw[t, qq] = 0 if c == 0 else max(128, c)
        if Lraw[t].sum() == 0:
            Lraw[t, 0] = 128  # guarantee >=1 chunk so PSUM gets a start matmul

    groups = [list(range(i, min(i + GROUP_TILES, T)))
              for i in range(0, T, GROUP_TILES)]

    nG = len(groups)
    run_start = [[0] * NQ for _ in range(nG)]
    run_len = [[0] * NQ for _ in range(nG)]
    run_valid16 = [[0] * NQ for _ in range(nG)]
    bucket_ofs = np.zeros((T, NQ), dtype=np.int64)  # global slot offset
    ofs = 0
    for gi, g in enumerate(groups):
        for qq in range(NQ):
            run_start[gi][qq] = ofs
            cur = ofs
            for t in g:
                bucket_ofs[t, qq] = cur
                cur += Lraw[t, qq]
            valid = cur - ofs
            # gather windows must be multiples of 128 rows: a window with
            # <128 rows leaves SDMA engines descriptor-less -> sem hang
            run_len[gi][qq] = _ceil(valid, 128)
            run_valid16[gi][qq] = run_len[gi][qq]
            ofs += run_len[gi][qq]
    S = ofs
    n_chunks = S // 128

    # chunk -> list of (tile, role); tile -> list of (chunk, role)
    tile_ops = [[] for _ in range(T)]
    chunk_roles = [[] for _ in range(n_chunks)]
    for gi, g in enumerate(groups):
        for qq in range(NQ):
            for t in g:
                L = Lraw[t, qq]
                if L == 0:
                    continue
                o = bucket_ofs[t, qq]
                c0, c1 = o // 128, (o + L - 1) // 128
                for c in range(c0, c1 + 1):
                    r = len(chunk_roles[c])
                    assert r < 2, "bucket <128 slots broke 2-role invariant"
                    chunk_roles[c].append(t)
                    tile_ops[t].append((int(c), r))

    per_core = []
    for c in range(N_CORES):
        sel = core == c
        s_c, t_c, q_c, dl_c = src[sel], tile_[sel], q[sel], dstl[sel]
        t_c = inv_perm[c][t_c]           # tile -> slot
        dl_c = dl_c % P                  # dst-within-tile (slot-local now)
        order = np.lexsort((q_c, t_c))
        s_c, t_c, q_c, dl_c = s_c[order], t_c[order], q_c[order], dl_c[order]
        idx_slots = np.zeros(S, dtype=np.int16)
        dstv_slots = np.full(S, PAD_DSTV, dtype=np.float32)
        owner = np.full(S, -1, dtype=np.int64)  # which tile owns slot (pads too)
        for t in range(T):
            for qq in range(NQ):
                if Lraw[t, qq]:
                    o = bucket_ofs[t, qq]
                    owner[o:o + Lraw[t, qq]] = t
        kk = t_c * NQ + q_c
        bounds = np.flatnonzero(np.diff(kk)) + 1
        starts = np.concatenate(([0], bounds))
        ends = np.concatenate((bounds, [len(kk)]))
        for a, b in zip(starts, ends):
            t, qq = int(t_c[a]), int(q_c[a])
            o = bucket_ofs[t, qq]
            n = b - a
            assert n <= Lraw[t, qq]
            idx_slots[o:o + n] = (s_c[a:b] - qq * QROWS).astype(np.int16)
            dstv_slots[o:o + n] = dl_c[a:b].astype(np.float32)
        # dv2: [128, 2*n_chunks]: col 2c+r = dstv where slot owner == role-r
        # tile of chunk c, else 200
        dv2 = np.full((128, 2 * n_chunks), PAD_DSTV, dtype=np.float32)
        sv = dstv_slots.reshape(n_chunks, 128)
        ow = owner.reshape(n_chunks, 128)
        for ch in range(n_chunks):
            for r, t in enumerate(chunk_roles[ch]):
                m = ow[ch] == t
                dv2[m, 2 * ch + r] = sv[ch][m]
        per_core.append({"idxs": _wrap16(idx_slots), "dv2": dv2,
                         "perm": perm[c]})

    meta = {
        "groups": groups, "run_start": run_start, "run_len": run_len,
        "run_valid16": run_valid16, "tile_ops": tile_ops,
        "n_chunks": n_chunks, "S": S, "Lraw": Lraw,
    }
    return meta, per_core


def _host_check(meta, per_core, feature, src, dst, W, b):
    """Numpy simulation of the device pipeline; returns rel err vs reference."""
    table = feature.astype(ml_dtypes.bfloat16).astype(np.float32)
    outs = []
    for c in range(N_CORES):
        idxs = per_core[c]["idxs"]
        dv2 = per_core[c]["dv2"]
        # unwrap idx slots
        S = meta["S"]
        unw = idxs[:16, :].T.reshape(-1)[:S].astype(np.int64)
        X = np.zeros((S, D), dtype=np.float32)
        for gi, g in enumerate(meta["groups"]):
            for qq in range(NQ):
                st = meta["run_start"][gi][qq]
                v16 = meta["run_valid16"][gi][qq]
                rows = unw[st:st + v16] + qq * QROWS
                X[st:st + v16] = table[rows]
        h = np.zeros((T * 128, D), dtype=np.float32)
        for t in range(T):
            acc = np.zeros((128, D), dtype=np.float32)
            for (ch, r) in meta["tile_ops"][t]:
                o = dv2[:, 2 * ch + r]  # [128] dst-within-tile or 200
                Ochunk = (o[:, None] == np.arange(128)[None, :])
                acc += (X[ch * 128:(ch + 1) * 128].T.astype(ml_dtypes.bfloat16)
                        .astype(np.float32) @ Ochunk).T
            h[t * 128:(t + 1) * 128] = acc
        o3 = h[:NC] @ W.T + b
        outs.append(np.maximum(o3, 0.0))
    return np.concatenate(outs, axis=0)


def _build(meta):
    nc = bacc.Bacc("TRN2", target_bir_lowering=False, debug=False,
                   num_devices=N_CORES)
    n_chunks = meta["n_chunks"]
    S = meta["S"]
    table = nc.dram_tensor("table", [N_NODES, D], _bf16, kind="ExternalInput").ap()
    idxs_d = nc.dram_tensor("idxs", [128, S // 16], _i16, kind="ExternalInput").ap()
    dv2_d = nc.dram_tensor("dv2", [128, 2 * n_chunks], _f32,
                           kind="ExternalInput").ap()
    iota_d = nc.dram_tensor("iota", [128, 128], _bf16, kind="ExternalInput").ap()
    wt_d = nc.dram_tensor("wt", [128, 128], _f32, kind="ExternalInput").ap()
    bias_d = nc.dram_tensor("bias", [128, 128], _f32, kind="ExternalInput").ap()
    out_d = nc.dram_tensor("out", [T * P, D], _f32, kind="ExternalOutput").ap()

    eq = mybir.AluOpType.is_equal
    add = mybir.AluOpType.add

    groups = meta["groups"]
    run_start = meta["run_start"]
    run_len = meta["run_len"]
    run_valid16 = meta["run_valid16"]
    tile_ops = meta["tile_ops"]

    with tile.TileContext(nc) as tc:
        nc.gpsimd.load_library(mlp)
        with (
            tc.tile_pool(name="const", bufs=1) as cp,
            tc.tile_pool(name="xp", bufs=2) as xp,
            tc.tile_pool(name="dvp", bufs=2) as dvp,
            tc.tile_pool(name="idxp", bufs=4) as idxp,
            tc.tile_pool(name="op", bufs=4) as op_,
            tc.tile_pool(name="htp", bufs=2) as htp,
            tc.tile_pool(name="obp", bufs=2) as obp,
            tc.tile_pool(name="hps", bufs=2, space="PSUM") as hp,
            tc.tile_pool(name="o3ps", bufs=2, space="PSUM") as o3p,
        ):
            iota_t = cp.tile([128, 128], _bf16, tag="iota")
            nc.sync.dma_start(iota_t[:], iota_d[:])
            wt_t = cp.tile([128, 128], _f32, tag="wt")
            nc.sync.dma_start(wt_t[:], wt_d[:])
            bias_t = cp.tile([128, 128], _f32, tag="bias")
            nc.sync.dma_start(bias_t[:], bias_d[:])

            for gi, g in enumerate(groups):
                g0 = run_start[gi][0]               # group slot base
                nch_g = sum(run_len[gi][qq] for qq in range(NQ)) // 128
                chunk0 = g0 // 128
                X = xp.tile([128, nch_g, 128], _bf16, tag="X")
                dv = dvp.tile([128, 2 * nch_g], _f32, tag="dv")
                nc.sync.dma_start(
                    dv[:], dv2_d[:, 2 * chunk0: 2 * (chunk0 + nch_g)])
                for qq in range(NQ):
                    v16 = run_valid16[gi][qq]
                    if v16 == 0:
                        continue
                    st = run_start[gi][qq]
                    for p0 in range(0, v16, MAX_GATHER_IDXS):
                        n_p = min(MAX_GATHER_IDXS, v16 - p0)
                        sp = st + p0
                        it = idxp.tile([128, n_p // 16], _i16, tag="idx")
                        nc.sync.dma_start(
                            it[:], idxs_d[:, sp // 16: sp // 16 + n_p // 16])
                        cb = (sp - g0) // 128
                        ncw = _ceil(n_p, 128) // 128
                        nc.gpsimd.dma_gather(
                            X[:, cb:cb + ncw, :],
                            table[qq * QROWS:(qq + 1) * QROWS, :],
                            it[:], n_p, n_p, D)

                hpt = hp.tile([128, len(g) * 128], _f32, tag="h")
                for tl, t in enumerate(g):
                    ops = tile_ops[t]
                    for oi, (ch, r) in enumerate(ops):
                        gc = ch - chunk0
                        O = op_.tile([128, 128], _bf16, tag="O")
                        nc.vector.tensor_scalar(
                            O[:], iota_t[:], dv[:, 2 * gc + r: 2 * gc + r + 1],
                            None, eq)
                        nc.tensor.matmul(
                            hpt[:, tl * 128:(tl + 1) * 128],
                            lhsT=X[:, gc, :], rhs=O[:],
                            start=(oi == 0), stop=(oi == len(ops) - 1))
                    ht = htp.tile([128, 128], _f32, tag="ht")
                    nc.scalar.copy(ht[:], hpt[:, tl * 128:(tl + 1) * 128])
                    o3 = o3p.tile([128, 128], _f32, tag="o3")
                    nc.tensor.matmul(o3[:], lhsT=ht[:], rhs=wt_t[:],
                                     start=True, stop=True)
                    ob = obp.tile([128, 128], _f32, tag="ob")
                    nc.vector.tensor_tensor(ob[:], o3[:], bias_t[:], op=add)
                    nc.vector.tensor_scalar_max(ob[:], ob[:], 0.0)
                    r0 = t * P
                    nc.sync.dma_start(out_d[r0:r0 + P, :], ob[:, :])
    nc.compile()
    return nc


_CACHE = {}
LAST_RES = None


def _get_compiled(src, dst):
    key = (hash(src.tobytes()), hash(dst.tobytes()))
    if key not in _CACHE:
        meta, per_core = _prep(src, dst)
        nc = _build(meta)
        _CACHE.clear()
        _CACHE[key] = (nc, meta, per_core)
    return _CACHE[key]


def _run(feature, src, dst, W, b, trace=False):
    feature = np.asarray(feature, dtype=np.float32)
    src = np.asarray(src).astype(np.int64)
    dst = np.asarray(dst).astype(np.int64)
    W = np.asarray(W, dtype=np.float32)
    b = np.asarray(b, dtype=np.float32)

    nc, meta, per_core = _get_compiled(src, dst)

    table = feature.astype(ml_dtypes.bfloat16)
    iota = np.tile(np.arange(128, dtype=np.float32), (128, 1)).astype(
        ml_dtypes.bfloat16)
    wt = np.ascontiguousarray(W.T)
    bias = np.tile(b, (128, 1)).astype(np.float32)

    in_maps = []
    for c in range(N_CORES):
        in_maps.append({
            "table": table,
            "idxs": per_core[c]["idxs"],
            "dv2": per_core[c]["dv2"],
            "iota": iota,
            "wt": wt,
            "bias": bias,
        })
    res = run_bass_kernel_spmd(nc, in_maps, core_ids=list(range(N_CORES)),
                               trace=trace)
    global LAST_RES
    LAST_RES = res
    outs = []
    for c in range(N_CORES):
        full = np.asarray(res.results[c]["out"], dtype=np.float32)  # [T*128, D]
        oc = np.empty((NC, D), dtype=np.float32)
        pm = per_core[c]["perm"]
        for lt in range(T):
            t = int(pm[lt])
            r0 = t * P
            n = min(P, NC - r0)
            oc[r0:r0 + n] = full[lt * P: lt * P + n]
        outs.append(oc)
    out = np.concatenate(outs, axis=0)
    return out.astype(np.float32), res.exec_time_ns


def kernel(feature, src, dst, W, b):
    return _run(feature, src, dst, W, b)[0]


def timed_run(inputs):
    return _run(**inputs, trace=True)[1]
